# revision 12
# baseline (speedup 1.0000x reference)
"""Trainium2 Bass kernel for nn_CrossAttention (B=2, N=2048, C=1024, H=16, D=64).

Sharding: 8 cores = 2 batches x 4 head-groups (4 heads each).
Each core computes its head-group's attention + a partial output projection;
the host sums the 4 partials per batch (bf16) and adds the bias.

Device pipeline per core:
  P1: q/k/v projections (bf16 matmuls), zero-mean folded into host-centered
      weights, variance via per-head ACT Square+accum (fp32, off the DVE),
      rstd via broadcast tensor_tensor, RoPE fused on DVE/gpsimd in bf16,
      bf16 PE transposes into head-paired q^T/k^T tiles, gate projected in
      transposed layout (raw, sigmoid deferred).  V is copied into augmented
      [V | 1] tiles so P2's attn@V matmuls also produce softmax denominators.
      ACT tables are warmed at t=0 behind the initial DMA wait; non-critical
      weight/table loads are deferred into the kv loop on the scalar queue.
  P2: per q-block of 512 tokens x 2 head-pairs.  Per key-chunk: one paired
      score matmul (row-split heads, two PSUM banks), exp via ACT Exp
      (10/16) or a DVE exp2 bit-trick (6/16), then two full-rate single
      matmuls with the augmented V (M=65) into per-head PSUM banks: row 64
      accumulates the softmax denominator for free -- no ones-matmuls.
      Gating: per-head reciprocal of the PSUM denominator row, K=1
      halves-matmul broadcast of 0.5/dn, (tanh+1)*rbc fused on DVE; head-1
      results are DMA-shifted to partitions 64-127 of A.  Output projection
      is interleaved into the next block's pair-0 chunk loop; stores split
      across both HWDGE queues per 512-column half.
"""

import os
import sys
import numpy as np

for _p in ("/opt/trn_rl_repo", "/opt/pypackages"):
    if _p not in sys.path:
        sys.path.insert(0, _p)

B, N, C = 2, 2048, 1024
H, D = 16, 64
HG = 4            # heads per core
NCH = 16          # token chunks of 128
KTC = 16          # key chunks of 128
EPS = 1e-6

# fast-exp: exp(s*0.125) ~= bitcast_bf16(int16(s*FE_A + FE_B))
FE_A = float(0.125 * np.log2(np.e) * 128.0)
FE_B = float(127.0 * 128.0 - 5.0)
APPROX_KC = (1, 3, 5, 7, 9, 11, 13, 15)   # DVE fast-exp chunks

_PROG = None      # cached compiled Bass program
LAST_EXEC_NS = None
LAST_PROFILE = None


def _build_program():
    import concourse.bass as bass
    import concourse.bacc as bacc
    import concourse.tile as tile
    import concourse.mybir as mybir

    F32 = mybir.dt.float32
    BF = mybir.dt.bfloat16
    I16 = mybir.dt.int16
    AF = mybir.ActivationFunctionType
    OP = mybir.AluOpType

    nc = bacc.Bacc("TRN2", target_bir_lowering=False, debug=False, num_devices=8)

    xT = nc.dram_tensor("xT", [128, 4, 8, 512], BF, kind="ExternalInput")
    ctxT = nc.dram_tensor("ctxT", [128, 4, 8, 512], BF, kind="ExternalInput")
    wq = nc.dram_tensor("wq", [128, 8, 256], BF, kind="ExternalInput")
    wg = nc.dram_tensor("wg", [128, 8, 256], BF, kind="ExternalInput")
    wkv = nc.dram_tensor("wkv", [128, 8, 512], BF, kind="ExternalInput")
    wo = nc.dram_tensor("wo", [128, 2, 1024], BF, kind="ExternalInput")
    cosq = nc.dram_tensor("cosq", [128, NCH, D], BF, kind="ExternalInput")
    ssinq = nc.dram_tensor("ssinq", [128, NCH, D], BF, kind="ExternalInput")
    cosk = nc.dram_tensor("cosk", [128, NCH, D], BF, kind="ExternalInput")
    ssink = nc.dram_tensor("ssink", [128, NCH, D], BF, kind="ExternalInput")
    part = nc.dram_tensor("part", [N, C], BF, kind="ExternalOutput")

    def bcast4(ap):
        # [128, 64] -> [128, 4, 64] with step-0 middle dim (read-broadcast)
        return bass.AP(tensor=ap.tensor, offset=ap.offset,
                       ap=[ap.ap[0], [0, 4], ap.ap[1]])

    def bcast64(ap):
        # [128, 4] -> [128, 4, 64] with step-0 last dim (per-head scalar)
        return bass.AP(tensor=ap.tensor, offset=ap.offset,
                       ap=[ap.ap[0], ap.ap[1], [0, 64]])

    def swap_view(ap):
        # ap: [128, 4, 64] contiguous -> per head read order d+32..d+63, d..d+31
        p, hdim, ddim = ap.ap
        return bass.AP(tensor=ap.tensor, offset=ap.offset + 32 * ddim[0],
                       ap=[p, hdim, [-32 * ddim[0], 2], [ddim[0], 32]])

    with tile.TileContext(nc) as tc:
        import contextlib
        with contextlib.ExitStack() as ctx:
            singles = ctx.enter_context(tc.tile_pool(name="singles", bufs=1))
            slices = ctx.enter_context(tc.tile_pool(name="slices", bufs=3))
            work = ctx.enter_context(tc.tile_pool(name="work", bufs=6))
            persist = ctx.enter_context(tc.tile_pool(name="persist", bufs=1))
            exps_p = ctx.enter_context(tc.tile_pool(name="exps", bufs=6))
            gat_p = ctx.enter_context(tc.tile_pool(name="gat", bufs=3))

            # ---- consts + ACT table warmup behind the initial DMA wait ----
            eps_sb = singles.tile([128, 1], F32)
            nc.vector.memset(eps_sb, EPS)
            halves = singles.tile([128, 64], BF)
            nc.vector.memset(halves, 1.0)
            scr1 = singles.tile([128, 1], F32)
            for f in (AF.Square, AF.Sqrt, AF.Copy, AF.Tanh, AF.Exp):
                nc.scalar.activation(out=scr1, in_=eps_sb, func=f)

            # ---- loads.  KV-path deps on the scalar HWDGE queue, context
            # token slices on the sync queue; everything else deferred.
            wkv_sb = singles.tile([128, 8, 512], BF)
            ck_sb = singles.tile([128, NCH, D], BF)
            sk_sb = singles.tile([128, NCH, D], BF)
            c_sl0 = singles.tile([128, 8, 512], BF)
            for h in range(2):
                hs = slice(4 * h, 4 * h + 4)
                nc.scalar.dma_start(out=wkv_sb[:, hs], in_=wkv.ap()[:, hs])
                nc.sync.dma_start(out=c_sl0[:, hs], in_=ctxT.ap()[:, 0, hs])
            nc.scalar.dma_start(out=ck_sb, in_=cosk.ap())
            nc.scalar.dma_start(out=sk_sb, in_=ssink.ap())

            wq_sb = singles.tile([128, 8, 256], BF)
            cq_sb = singles.tile([128, NCH, D], BF)
            sq_sb = singles.tile([128, NCH, D], BF)
            wg_sb = singles.tile([128, 8, 256], BF)
            wo_sb = singles.tile([128, 2, 1024], BF)
            deferred = {1: [(wq_sb, wq)], 3: [(cq_sb, cosq), (sq_sb, ssinq)],
                        5: [(wg_sb, wg)], 8: [(wo_sb, wo)]}

            from concourse.masks import make_identity
            ident = singles.tile([128, 128], BF)
            make_identity(nc, ident)

            # ---- persistent intermediates ----
            pairQ = [persist.tile([128, N], BF, tag=f"pairQ{p}",
                                  name=f"pairQ{p}") for p in range(2)]
            pairK = [persist.tile([128, N], BF, tag=f"pairK{p}",
                                  name=f"pairK{p}") for p in range(2)]
            # augmented V: per (kchunk, pair, head) [V | 1] -> denominator row
            v65 = persist.tile([128, KTC, 2, 2, 65], BF, tag="v65")
            nc.vector.memset(v65, 1.0)
            graw = persist.tile([128, 2, N], BF, tag="graw")
            A_sb = persist.tile([128, 2, N], BF, tag="A_sb")

            # ================= P1: projections / norm / rope / transposes ====
            with tc.tile_pool(name="psA", bufs=4, space="PSUM") as psA, \
                 tc.tile_pool(name="psT", bufs=4, space="PSUM") as psT:

                pend_t = []  # (qr, i, dst_pair) transposes delayed 2 chunks

                def flush_transpose(qr, i, dst_pair):
                    # PE transpose: heads (2p, 2p+1) -> pair tile slice (bf16)
                    for p in range(2):
                        pst = psT.tile([128, 128], BF, tag="tp")
                        nc.tensor.transpose(
                            pst,
                            qr[:, 2 * p:2 * p + 2, :].rearrange("p a b -> p (a b)"),
                            ident)
                        nc.scalar.activation(
                            out=dst_pair[p][:, i * 128:(i + 1) * 128],
                            in_=pst, func=AF.Copy)

                def qk_path(sl, ns, i, w_rhs, wcols, cos_t, sin_t, dst_pair):
                    """Project+norm+rope chunk i of q or k; transpose deferred."""
                    ps = psA.tile([128, 512], F32, tag="proj")
                    for c in range(8):
                        nc.tensor.matmul(ps[:, :wcols],
                                         sl[:, c, ns * 128:(ns + 1) * 128],
                                         w_rhs(c),
                                         start=(c == 0), stop=(c == 7))
                    qpart = ps[:, 0:256]
                    # variance (zero-mean folded into host-centered weights)
                    sqv = work.tile([128, 256], BF, tag="sq")
                    nc.scalar.activation(out=sqv, in_=qpart, func=AF.Square)
                    ssum = work.tile([128, 4], BF, tag="ssum")
                    with nc.allow_low_precision("rmsnorm stats tolerate bf16"):
                        nc.vector.tensor_reduce(
                            out=ssum, in_=sqv.rearrange("p (h d) -> p h d", h=4),
                            axis=mybir.AxisListType.X, op=OP.add)
                    sdev = work.tile([128, 4], F32, tag="sdev")
                    nc.scalar.activation(out=sdev, in_=ssum, func=AF.Sqrt,
                                         scale=1.0 / 64.0, bias=eps_sb)
                    rstd = work.tile([128, 4], F32, tag="rstd")
                    nc.vector.reciprocal(out=rstd, in_=sdev)
                    qs = work.tile([128, 4, 64], BF, tag="qs")
                    nc.vector.tensor_tensor(
                        out=qs, in0=qpart.rearrange("p (h d) -> p h d", h=4),
                        in1=bcast64(rstd), op=OP.mult)
                    # rope: qr = qs*cos + swap(qs)*ssin (sign folded in ssin)
                    t1 = work.tile([128, 4, 64], BF, tag="t1")
                    nc.vector.tensor_tensor(out=t1, in0=qs, in1=bcast4(cos_t),
                                            op=OP.mult)
                    t2 = work.tile([128, 4, 64], BF, tag="t2")
                    nc.gpsimd.tensor_tensor(out=t2, in0=swap_view(qs),
                                            in1=bcast4(sin_t), op=OP.mult)
                    qr = work.tile([128, 4, 64], BF, tag="qr")
                    nc.gpsimd.tensor_tensor(out=qr, in0=t1, in1=t2, op=OP.add)
                    pend_t.append((qr, i, dst_pair))
                    if len(pend_t) > 3:
                        flush_transpose(*pend_t.pop(0))
                        flush_transpose(*pend_t.pop(0))
                    return ps

                # K/V path over all 16 chunks
                csls = [c_sl0]
                for qc in range(1, 4):
                    c_sl = slices.tile([128, 8, 512], BF, tag="slice")
                    nc.sync.dma_start(out=c_sl, in_=ctxT.ap()[:, qc])
                    csls.append(c_sl)
                for qc in range(4):
                    c_sl = csls[qc]
                    for ns in range(4):
                        j = qc * 4 + ns
                        ps = qk_path(c_sl, ns, j,
                                     lambda c: wkv_sb[:, c, :], 512,
                                     ck_sb[:, j, :], sk_sb[:, j, :], pairK)
                        nc.vector.tensor_copy(
                            out=v65[:, j, :, :, 0:64],
                            in_=ps[:, 256:512]
                                .rearrange("p (a h d) -> p a h d", a=2, h=2))
                        for dst, src in deferred.get(j, ()):
                            nc.scalar.dma_start(out=dst, in_=src.ap())

                # Q path + raw gate over all 16 chunks
                for qc in range(4):
                    x_sl = slices.tile([128, 8, 512], BF, tag="slice")
                    nc.sync.dma_start(out=x_sl, in_=xT.ap()[:, qc])
                    for ns in range(4):
                        i = qc * 4 + ns
                        qk_path(x_sl, ns, i,
                                lambda c: wq_sb[:, c, :], 256,
                                cq_sb[:, i, :], sq_sb[:, i, :], pairQ)
                    # gate projection, transposed layout, raw (sigmoid later)
                    for gfc in range(2):
                        psg = psA.tile([128, 512], F32, tag="proj")
                        for c in range(8):
                            nc.tensor.matmul(
                                psg, wg_sb[:, c, gfc * 128:(gfc + 1) * 128],
                                x_sl[:, c, :], start=(c == 0), stop=(c == 7))
                        nc.scalar.activation(
                            out=graw[:, gfc, qc * 512:(qc + 1) * 512], in_=psg,
                            func=AF.Copy)
                for e in pend_t:
                    flush_transpose(*e)
                del pend_t[:]
                # re-warm P2 tables
                nc.scalar.activation(out=scr1, in_=eps_sb, func=AF.Tanh)
                nc.scalar.activation(out=scr1, in_=eps_sb, func=AF.Exp)

            # ================= P2: attention + gating + out-proj =============
            with tc.tile_pool(name="psSC", bufs=2, space="PSUM") as psSC, \
                 tc.tile_pool(name="psAO", bufs=1, space="PSUM") as psAO, \
                 tc.tile_pool(name="psX", bufs=2, space="PSUM") as psX:

                def emit_outproj_nk(nk, tail=False):
                    # output projection + store for one 128-token row block
                    n1s = slice(nk * 128, (nk + 1) * 128)
                    ev = gat_p.tile([128, 1024], BF, tag="ev")
                    for oc in range(2):
                        po = psX.tile([128, 512], F32, tag="px")
                        for fc in range(2):
                            nc.tensor.matmul(
                                po, A_sb[:, fc, n1s],
                                wo_sb[:, fc, oc * 512:(oc + 1) * 512],
                                start=(fc == 0), stop=(fc == 1))
                        ocs = slice(oc * 512, (oc + 1) * 512)
                        nc.scalar.activation(out=ev[:, ocs], in_=po,
                                             func=AF.Copy)
                        (nc.scalar if (tail and oc == 1) else nc.sync)\
                            .dma_start(out=part.ap()[n1s, ocs], in_=ev[:, ocs])

                for qc in range(4):
                    qsl = slice(qc * 512, (qc + 1) * 512)
                    for p in range(2):
                        # tanh(graw/2) early; ACT slots it between exps
                        gs = gat_p.tile([128, 512], F32, tag="gs")
                        nc.scalar.activation(out=gs, in_=graw[:, p, qsl],
                                             func=AF.Tanh, scale=0.5)
                        gs1 = gat_p.tile([64, 512], F32, tag="gs1")
                        nc.sync.dma_start(out=gs1, in_=gs[64:128, :])

                        aoT = psAO.tile([128, 1024], F32, tag="aoT")
                        aoA = aoT[:, 0:512]
                        aoB = aoT[:, 512:1024]
                        pend = []

                        def flush_attn(eS, k, aoA=aoA, aoB=aoB, p=p):
                            st = (k == 0)
                            sp = (k == KTC - 1)
                            nc.tensor.matmul(aoA[0:65, :],
                                             v65[:, k, p, 0, :], eS[:, 0, :],
                                             start=st, stop=sp,
                                             skip_group_check=True)
                            nc.tensor.matmul(aoB[0:65, :],
                                             v65[:, k, p, 1, :], eS[:, 1, :],
                                             start=st, stop=sp,
                                             skip_group_check=True)

                        po_at = {3: 2 * p, 9: 2 * p + 1} if qc > 0 else {}
                        for k in range(KTC):
                            ksl = slice(k * 128, (k + 1) * 128)
                            ps = psSC.tile([128, 1024], F32, tag="sc")
                            nc.tensor.matmul(ps[:, 0:512],
                                             pairK[p][0:64, ksl],
                                             pairQ[p][0:64, qsl],
                                             start=True, stop=True,
                                             tile_position=(0, 0))
                            nc.tensor.matmul(ps[:, 512:1024],
                                             pairK[p][64:128, ksl],
                                             pairQ[p][64:128, qsl],
                                             start=True, stop=True,
                                             tile_position=(64, 0))
                            eS = exps_p.tile([128, 2, 512], BF, tag="expS")
                            if k in APPROX_KC:
                                nc.vector.tensor_scalar(
                                    out=eS.bitcast(I16)
                                        .rearrange("p a b -> p (a b)"),
                                    in0=ps, scalar1=FE_A, scalar2=FE_B,
                                    op0=OP.mult, op1=OP.add)
                            else:
                                nc.scalar.activation(
                                    out=eS.rearrange("p a b -> p (a b)"),
                                    in_=ps, func=AF.Exp, scale=0.125)
                            pend.append((eS, k))
                            if len(pend) > 3:
                                flush_attn(*pend.pop(0))
                            if k in po_at:
                                emit_outproj_nk(4 * (qc - 1) + po_at[k])
                        for e in pend:
                            flush_attn(*e)

                        # gating: A = ao * (tanh(g/2)+1) / dn  (0.5 in Wo)
                        dnb = gat_p.tile([1, 2, 512], BF, tag="dnb")
                        nc.scalar.activation(out=dnb, in_=aoT[64:65, :]
                                             .rearrange("p (a b) -> p a b", a=2),
                                             func=AF.Copy)
                        rb0 = psX.tile([128, 512], F32, tag="px")
                        nc.tensor.matmul(rb0[0:64, :], halves[0:1, :],
                                         dnb[:, 0, :], start=True, stop=True)
                        rb1 = psX.tile([128, 512], F32, tag="px")
                        nc.tensor.matmul(rb1[0:64, :], halves[0:1, :],
                                         dnb[:, 1, :], start=True, stop=True)
                        rec0 = gat_p.tile([64, 512], F32, tag="rec0")
                        nc.vector.reciprocal_approx_fast(out=rec0,
                                                         in_=rb0[0:64, :])
                        m0 = gat_p.tile([64, 512], F32, tag="m0")
                        nc.vector.scalar_tensor_tensor(
                            out=m0, in0=gs[0:64, :], scalar=1.0,
                            in1=rec0, op0=OP.add, op1=OP.mult)
                        nc.vector.tensor_tensor(out=A_sb[0:64, p, qsl],
                                                in0=aoA[0:64, :], in1=m0,
                                                op=OP.mult)
                        rec1 = gat_p.tile([64, 512], F32, tag="rec1")
                        nc.vector.reciprocal_approx_fast(out=rec1,
                                                         in_=rb1[0:64, :])
                        m1 = gat_p.tile([64, 512], F32, tag="m1")
                        nc.vector.scalar_tensor_tensor(
                            out=m1, in0=gs1, scalar=1.0,
                            in1=rec1, op0=OP.add, op1=OP.mult)
                        At = gat_p.tile([64, 512], BF, tag="At")
                        nc.vector.tensor_tensor(out=At, in0=aoB[0:64, :],
                                                in1=m1, op=OP.mult)
                        nc.sync.dma_start(out=A_sb[64:128, p, qsl], in_=At)

                for nk in range(12, 16):
                    emit_outproj_nk(nk, tail=True)

    nc.compile()
    return nc


def _prep_core(inputs, b, g, bf16):
    x = np.asarray(inputs["x"][b], dtype=np.float32)
    ctx = np.asarray(inputs["context"][b], dtype=np.float32)
    Wq = np.asarray(inputs["Wq"], dtype=np.float32).reshape(H, 2 * D, C)
    Wkv = np.asarray(inputs["Wkv"], dtype=np.float32).reshape(H, 2 * D, C)
    Wo = np.asarray(inputs["Wo"], dtype=np.float32)
    cos = np.asarray(inputs["cos"][b], dtype=np.float32)
    sin = np.asarray(inputs["sin"][b], dtype=np.float32)
    qw = np.asarray(inputs["q_norm_w"], dtype=np.float32)
    kw = np.asarray(inputs["k_norm_w"], dtype=np.float32)

    hs = slice(HG * g, HG * g + HG)
    qr = Wq[hs, :D, :]
    qr = qr - qr.mean(axis=1, keepdims=True)
    gr = Wq[hs, D:, :]
    kr = Wkv[hs, :D, :]
    kr = kr - kr.mean(axis=1, keepdims=True)
    vr = Wkv[hs, D:, :]

    sgn = np.where(np.arange(D) < D // 2, -1.0, 1.0).astype(np.float32)
    wswap = lambda w: np.concatenate([w[D // 2:], w[:D // 2]])

    def pmajor(w, cols):
        # [cols, C] weight -> transposed, partition-major [128, 8, cols]
        return np.ascontiguousarray(
            w.reshape(cols, C).T.reshape(8, 128, cols).transpose(1, 0, 2))

    def tokmajor(t):
        # [C, N] -> [128, 4, 8, 512]: partition, q-block, c-chunk, token
        return np.ascontiguousarray(
            t.reshape(8, 128, 4, 512).transpose(1, 2, 0, 3))

    def tabs(t):
        # [N, D] -> [128, 16, D] bf16
        return np.ascontiguousarray(
            t.reshape(16, 128, D).transpose(1, 0, 2)).astype(bf16)

    return {
        "xT": tokmajor(x.T).astype(bf16),
        "ctxT": tokmajor(ctx.T).astype(bf16),
        "wq": pmajor(qr, 256).astype(bf16),
        "wg": pmajor(gr, 256).astype(bf16),
        "wkv": pmajor(
            np.concatenate([kr.reshape(HG * D, C), vr.reshape(HG * D, C)], 0),
            512).astype(bf16),
        "wo": np.ascontiguousarray(
            0.5 * Wo[:, 256 * g:256 * (g + 1)].T.reshape(2, 128, C)
            .transpose(1, 0, 2)).astype(bf16),
        "cosq": tabs(cos * qw[None, :]),
        "ssinq": tabs(sin * sgn[None, :] * wswap(qw)[None, :]),
        "cosk": tabs(cos * kw[None, :]),
        "ssink": tabs(sin * sgn[None, :] * wswap(kw)[None, :]),
    }


def kernel(**inputs):
    global _PROG, LAST_EXEC_NS, LAST_PROFILE
    import ml_dtypes
    bf16 = ml_dtypes.bfloat16

    if _PROG is None:
        _PROG = _build_program()
    nc = _PROG

    in_maps = [_prep_core(inputs, core // 4, core % 4, bf16) for core in range(8)]

    trace = bool(os.environ.get("BASS_KERNEL_TRACE"))
    kw = {}
    if trace:
        import types
        from trn_agent_boot.trn_boot import _ntff_profile_via_ctypes
        hook = _ntff_profile_via_ctypes('/opt/axon/libaxon_pjrt.so')
        mod = types.ModuleType('antenv.axon_hooks')
        mod.get_axon_ntff_profile_hook = lambda: hook
        sys.modules['antenv.axon_hooks'] = mod
        from concourse import bass_utils
        bass_utils.upload_artifacts = lambda tmpdir: tmpdir
        kw = dict(trace=True, tmpdir=os.environ.get("BASS_KERNEL_TRACE_DIR"))

    from concourse.bass_utils import run_bass_kernel_spmd
    res = run_bass_kernel_spmd(nc, in_maps, core_ids=list(range(8)), **kw)
    LAST_EXEC_NS = res.exec_time_ns
    LAST_PROFILE = res.profile_json

    bo = np.asarray(inputs["bo"], dtype=np.float32)
    out = np.zeros((B, N, C), dtype=np.float32)
    for core in range(8):
        out[core // 4] += np.asarray(res.results[core]["part"],
                                     dtype=np.float32)
    out += bo[None, None, :]
    return out


# revision 13
# speedup vs baseline: 1.2376x; 1.2376x over previous
"""Trainium2 Bass kernel for nn_CrossAttention (B=2, N=2048, C=1024, H=16, D=64).

Sharding: 8 cores = 2 batches x 4 head-groups (4 heads each).
Each core computes its head-group's attention + a partial output projection;
the host sums the 4 partials per batch (bf16) and adds the bias.

Device pipeline per core:
  P1: q/k/v projections (bf16 matmuls), zero-mean folded into host-centered
      weights, variance via per-head ACT Square+accum (fp32, off the DVE),
      rstd via broadcast tensor_tensor, RoPE fused on DVE/gpsimd in bf16,
      bf16 PE transposes into head-paired q^T/k^T tiles, gate projected in
      transposed layout (raw, sigmoid deferred).  V is copied into augmented
      [V | 1] tiles so P2's attn@V matmuls also produce softmax denominators.
      ACT tables are warmed at t=0 behind the initial DMA wait; non-critical
      weight/table loads are deferred into the kv loop on the scalar queue.
  P2: per q-block of 512 tokens x 2 head-pairs.  Per key-chunk: one paired
      score matmul (row-split heads, two PSUM banks), exp via ACT Exp
      (10/16) or a DVE exp2 bit-trick (6/16), then two full-rate single
      matmuls with the augmented V (M=65) into per-head PSUM banks: row 64
      accumulates the softmax denominator for free -- no ones-matmuls.
      Gating: per-head reciprocal of the PSUM denominator row, K=1
      halves-matmul broadcast of 0.5/dn, (tanh+1)*rbc fused on DVE; head-1
      results are DMA-shifted to partitions 64-127 of A.  Output projection
      is interleaved into the next block's pair-0 chunk loop; stores split
      across both HWDGE queues per 512-column half.
"""

import os
import sys
import numpy as np

for _p in ("/opt/trn_rl_repo", "/opt/pypackages"):
    if _p not in sys.path:
        sys.path.insert(0, _p)

B, N, C = 2, 2048, 1024
H, D = 16, 64
HG = 4            # heads per core
NCH = 16          # token chunks of 128
KTC = 16          # key chunks of 128
EPS = 1e-6

# fast-exp: exp(s*0.125) ~= bitcast_bf16(int16(s*FE_A + FE_B))
FE_A = float(0.125 * np.log2(np.e) * 128.0)
FE_B = float(127.0 * 128.0 - 5.0)
APPROX_KC = (1, 3, 5, 7, 9, 11, 13, 15)   # DVE fast-exp chunks

_PROG = None      # cached compiled Bass program
LAST_EXEC_NS = None
LAST_PROFILE = None


def _build_program():
    import concourse.bass as bass
    import concourse.bacc as bacc
    import concourse.tile as tile
    import concourse.mybir as mybir

    F32 = mybir.dt.float32
    BF = mybir.dt.bfloat16
    I16 = mybir.dt.int16
    AF = mybir.ActivationFunctionType
    OP = mybir.AluOpType

    nc = bacc.Bacc("TRN2", target_bir_lowering=False, debug=False, num_devices=8)

    xT = nc.dram_tensor("xT", [128, 4, 8, 512], BF, kind="ExternalInput")
    ctxT = nc.dram_tensor("ctxT", [128, 4, 8, 512], BF, kind="ExternalInput")
    wq = nc.dram_tensor("wq", [128, 8, 256], BF, kind="ExternalInput")
    wg = nc.dram_tensor("wg", [128, 8, 256], BF, kind="ExternalInput")
    wkv = nc.dram_tensor("wkv", [128, 8, 512], BF, kind="ExternalInput")
    wo = nc.dram_tensor("wo", [128, 2, 1024], BF, kind="ExternalInput")
    cosq = nc.dram_tensor("cosq", [128, NCH, D], BF, kind="ExternalInput")
    ssinq = nc.dram_tensor("ssinq", [128, NCH, D], BF, kind="ExternalInput")
    cosk = nc.dram_tensor("cosk", [128, NCH, D], BF, kind="ExternalInput")
    ssink = nc.dram_tensor("ssink", [128, NCH, D], BF, kind="ExternalInput")
    part = nc.dram_tensor("part", [N, C], BF, kind="ExternalOutput")

    def bcast4(ap):
        # [128, 64] -> [128, 4, 64] with step-0 middle dim (read-broadcast)
        return bass.AP(tensor=ap.tensor, offset=ap.offset,
                       ap=[ap.ap[0], [0, 4], ap.ap[1]])

    def bcast64(ap):
        # [128, 4] -> [128, 4, 64] with step-0 last dim (per-head scalar)
        return bass.AP(tensor=ap.tensor, offset=ap.offset,
                       ap=[ap.ap[0], ap.ap[1], [0, 64]])

    def swap_view(ap):
        # ap: [128, 4, 64] contiguous -> per head read order d+32..d+63, d..d+31
        p, hdim, ddim = ap.ap
        return bass.AP(tensor=ap.tensor, offset=ap.offset + 32 * ddim[0],
                       ap=[p, hdim, [-32 * ddim[0], 2], [ddim[0], 32]])

    with tile.TileContext(nc) as tc:
        import contextlib
        with contextlib.ExitStack() as ctx:
            singles = ctx.enter_context(tc.tile_pool(name="singles", bufs=1))
            slices = ctx.enter_context(tc.tile_pool(name="slices", bufs=3))
            work = ctx.enter_context(tc.tile_pool(name="work", bufs=6))
            persist = ctx.enter_context(tc.tile_pool(name="persist", bufs=1))
            exps_p = ctx.enter_context(tc.tile_pool(name="exps", bufs=6))
            gat_p = ctx.enter_context(tc.tile_pool(name="gat", bufs=3))

            # ---- consts + ACT table warmup behind the initial DMA wait ----
            eps_sb = singles.tile([128, 1], F32)
            nc.vector.memset(eps_sb, EPS)
            halves = singles.tile([128, 64], BF)
            nc.vector.memset(halves, 1.0)
            scr1 = singles.tile([128, 1], F32)
            for f in (AF.Square, AF.Sqrt, AF.Copy, AF.Tanh, AF.Exp):
                nc.scalar.activation(out=scr1, in_=eps_sb, func=f)

            # ---- loads.  KV-path deps on the scalar HWDGE queue, context
            # token slices on the sync queue; everything else deferred.
            wkv_sb = singles.tile([128, 8, 512], BF)
            ck_sb = singles.tile([128, NCH, D], BF)
            sk_sb = singles.tile([128, NCH, D], BF)
            c_sl0 = singles.tile([128, 8, 512], BF)
            for h in range(2):
                hs = slice(4 * h, 4 * h + 4)
                nc.scalar.dma_start(out=wkv_sb[:, hs], in_=wkv.ap()[:, hs])
                nc.sync.dma_start(out=c_sl0[:, hs], in_=ctxT.ap()[:, 0, hs])
            nc.scalar.dma_start(out=ck_sb, in_=cosk.ap())
            nc.scalar.dma_start(out=sk_sb, in_=ssink.ap())

            wq_sb = singles.tile([128, 8, 256], BF)
            cq_sb = singles.tile([128, NCH, D], BF)
            sq_sb = singles.tile([128, NCH, D], BF)
            wg_sb = singles.tile([128, 8, 256], BF)
            wo_sb = singles.tile([128, 2, 1024], BF)
            deferred = {1: [(wq_sb, wq)], 3: [(cq_sb, cosq), (sq_sb, ssinq)],
                        5: [(wg_sb, wg)], 8: [(wo_sb, wo)]}

            from concourse.masks import make_identity
            ident = singles.tile([128, 128], BF)
            make_identity(nc, ident)

            # ---- persistent intermediates ----
            pairQ = [persist.tile([128, N], BF, tag=f"pairQ{p}",
                                  name=f"pairQ{p}") for p in range(2)]
            pairK = [persist.tile([128, N], BF, tag=f"pairK{p}",
                                  name=f"pairK{p}") for p in range(2)]
            # augmented V: per (kchunk, pair, head) [V | 1] -> denominator row
            v65 = persist.tile([128, KTC, 2, 2, 65], BF, tag="v65")
            nc.vector.memset(v65, 1.0)
            graw = persist.tile([128, 2, N], BF, tag="graw")
            A_sb = persist.tile([128, 2, N], BF, tag="A_sb")

            # ================= P1: projections / norm / rope / transposes ====
            with tc.tile_pool(name="psA", bufs=4, space="PSUM") as psA, \
                 tc.tile_pool(name="psT", bufs=4, space="PSUM") as psT:

                pend_t = []  # (qr, i, dst_pair) transposes delayed 2 chunks

                def flush_transpose(qr, i, dst_pair):
                    # PE transpose: heads (2p, 2p+1) -> pair tile slice (bf16)
                    for p in range(2):
                        pst = psT.tile([128, 128], BF, tag="tp")
                        nc.tensor.transpose(
                            pst,
                            qr[:, 2 * p:2 * p + 2, :].rearrange("p a b -> p (a b)"),
                            ident)
                        nc.scalar.activation(
                            out=dst_pair[p][:, i * 128:(i + 1) * 128],
                            in_=pst, func=AF.Copy)

                def qk_path(sl, ns, i, w_rhs, wcols, cos_t, sin_t, dst_pair):
                    """Project+norm+rope chunk i of q or k; transpose deferred."""
                    ps = psA.tile([128, 512], F32, tag="proj")
                    for c in range(8):
                        nc.tensor.matmul(ps[:, :wcols],
                                         sl[:, c, ns * 128:(ns + 1) * 128],
                                         w_rhs(c),
                                         start=(c == 0), stop=(c == 7))
                    qpart = ps[:, 0:256]
                    # variance (zero-mean folded into host-centered weights)
                    sqv = work.tile([128, 256], BF, tag="sq")
                    nc.scalar.activation(out=sqv, in_=qpart, func=AF.Square)
                    ssum = work.tile([128, 4], BF, tag="ssum")
                    with nc.allow_low_precision("rmsnorm stats tolerate bf16"):
                        nc.vector.tensor_reduce(
                            out=ssum, in_=sqv.rearrange("p (h d) -> p h d", h=4),
                            axis=mybir.AxisListType.X, op=OP.add)
                    sdev = work.tile([128, 4], F32, tag="sdev")
                    nc.scalar.activation(out=sdev, in_=ssum, func=AF.Sqrt,
                                         scale=1.0 / 64.0, bias=eps_sb)
                    rstd = work.tile([128, 4], F32, tag="rstd")
                    nc.vector.reciprocal(out=rstd, in_=sdev)
                    qs = work.tile([128, 4, 64], BF, tag="qs")
                    nc.vector.tensor_tensor(
                        out=qs, in0=qpart.rearrange("p (h d) -> p h d", h=4),
                        in1=bcast64(rstd), op=OP.mult)
                    # rope: qr = qs*cos + swap(qs)*ssin (sign folded in ssin)
                    t1 = work.tile([128, 4, 64], BF, tag="t1")
                    nc.vector.tensor_tensor(out=t1, in0=qs, in1=bcast4(cos_t),
                                            op=OP.mult)
                    t2 = work.tile([128, 4, 64], BF, tag="t2")
                    nc.gpsimd.tensor_tensor(out=t2, in0=swap_view(qs),
                                            in1=bcast4(sin_t), op=OP.mult)
                    qr = work.tile([128, 4, 64], BF, tag="qr")
                    nc.gpsimd.tensor_tensor(out=qr, in0=t1, in1=t2, op=OP.add)
                    pend_t.append((qr, i, dst_pair))
                    if len(pend_t) > 3:
                        flush_transpose(*pend_t.pop(0))
                        flush_transpose(*pend_t.pop(0))
                    return ps

                # K/V path over all 16 chunks
                csls = [c_sl0]
                for qc in range(1, 4):
                    c_sl = slices.tile([128, 8, 512], BF, tag="slice")
                    nc.sync.dma_start(out=c_sl, in_=ctxT.ap()[:, qc])
                    csls.append(c_sl)
                for qc in range(4):
                    c_sl = csls[qc]
                    for ns in range(4):
                        j = qc * 4 + ns
                        ps = qk_path(c_sl, ns, j,
                                     lambda c: wkv_sb[:, c, :], 512,
                                     ck_sb[:, j, :], sk_sb[:, j, :], pairK)
                        nc.vector.tensor_copy(
                            out=v65[:, j, :, :, 0:64],
                            in_=ps[:, 256:512]
                                .rearrange("p (a h d) -> p a h d", a=2, h=2))
                        for dst, src in deferred.get(j, ()):
                            nc.scalar.dma_start(out=dst, in_=src.ap())

                # Q path + raw gate over all 16 chunks
                for qc in range(4):
                    x_sl = slices.tile([128, 8, 512], BF, tag="slice")
                    nc.sync.dma_start(out=x_sl, in_=xT.ap()[:, qc])
                    for ns in range(4):
                        i = qc * 4 + ns
                        qk_path(x_sl, ns, i,
                                lambda c: wq_sb[:, c, :], 256,
                                cq_sb[:, i, :], sq_sb[:, i, :], pairQ)
                    # gate projection, transposed layout, raw (sigmoid later)
                    for gfc in range(2):
                        psg = psA.tile([128, 512], F32, tag="proj")
                        for c in range(8):
                            nc.tensor.matmul(
                                psg, wg_sb[:, c, gfc * 128:(gfc + 1) * 128],
                                x_sl[:, c, :], start=(c == 0), stop=(c == 7))
                        nc.scalar.activation(
                            out=graw[:, gfc, qc * 512:(qc + 1) * 512], in_=psg,
                            func=AF.Copy)
                for e in pend_t:
                    flush_transpose(*e)
                del pend_t[:]
                # re-warm P2 tables
                nc.scalar.activation(out=scr1, in_=eps_sb, func=AF.Tanh)
                nc.scalar.activation(out=scr1, in_=eps_sb, func=AF.Exp)

            # ================= P2: attention + gating + out-proj =============
            with tc.tile_pool(name="psSC", bufs=4, space="PSUM") as psSC, \
                 tc.tile_pool(name="psAO", bufs=1, space="PSUM") as psAO, \
                 tc.tile_pool(name="psX", bufs=2, space="PSUM") as psX:

                def emit_outproj_nk(nk, tail=False):
                    # output projection + store for one 128-token row block
                    n1s = slice(nk * 128, (nk + 1) * 128)
                    ev = gat_p.tile([128, 1024], BF, tag="ev")
                    for oc in range(2):
                        po = psX.tile([128, 512], F32, tag="px")
                        for fc in range(2):
                            nc.tensor.matmul(
                                po, A_sb[:, fc, n1s],
                                wo_sb[:, fc, oc * 512:(oc + 1) * 512],
                                start=(fc == 0), stop=(fc == 1))
                        ocs = slice(oc * 512, (oc + 1) * 512)
                        if oc == 0:
                            nc.vector.tensor_copy(out=ev[:, ocs], in_=po)
                        else:
                            nc.scalar.activation(out=ev[:, ocs], in_=po,
                                                 func=AF.Copy)
                        (nc.scalar if (tail and oc == 1) else nc.sync)\
                            .dma_start(out=part.ap()[n1s, ocs], in_=ev[:, ocs])

                for qc in range(4):
                    qsl = slice(qc * 512, (qc + 1) * 512)
                    for p in range(2):
                        # tanh(graw/2) early; ACT slots it between exps
                        gs = gat_p.tile([128, 512], F32, tag="gs")
                        nc.scalar.activation(out=gs, in_=graw[:, p, qsl],
                                             func=AF.Tanh, scale=0.5)
                        gs1 = gat_p.tile([64, 512], F32, tag="gs1")
                        nc.sync.dma_start(out=gs1, in_=gs[64:128, :])

                        aoT = psAO.tile([128, 1024], F32, tag="aoT")
                        aoA = aoT[:, 0:512]
                        aoB = aoT[:, 512:1024]
                        pend = []

                        def flush_attn(eS, k, aoA=aoA, aoB=aoB, p=p):
                            st = (k == 0)
                            sp = (k == KTC - 1)
                            nc.tensor.matmul(aoA[0:65, :],
                                             v65[:, k, p, 0, :], eS[:, 0, :],
                                             start=st, stop=sp,
                                             skip_group_check=True)
                            nc.tensor.matmul(aoB[0:65, :],
                                             v65[:, k, p, 1, :], eS[:, 1, :],
                                             start=st, stop=sp,
                                             skip_group_check=True)

                        po_at = {3: 2 * p, 9: 2 * p + 1} if qc > 0 else {}
                        for k in range(KTC):
                            ksl = slice(k * 128, (k + 1) * 128)
                            sA = psSC.tile([128, 512], F32, tag="sc")
                            sB = psSC.tile([128, 512], F32, tag="sc")
                            nc.tensor.matmul(sA,
                                             pairK[p][0:64, ksl],
                                             pairQ[p][0:64, qsl],
                                             start=True, stop=True,
                                             tile_position=(0, 0))
                            nc.tensor.matmul(sB,
                                             pairK[p][64:128, ksl],
                                             pairQ[p][64:128, qsl],
                                             start=True, stop=True,
                                             tile_position=(64, 0))
                            eS = exps_p.tile([128, 2, 512], BF, tag="expS")
                            nc.scalar.activation(
                                out=eS[:, 0, :], in_=sA,
                                func=AF.Exp, scale=0.125)
                            nc.vector.tensor_scalar(
                                out=eS[:, 1, :].bitcast(I16),
                                in0=sB, scalar1=FE_A, scalar2=FE_B,
                                op0=OP.mult, op1=OP.add)
                            pend.append((eS, k))
                            if len(pend) > 2:
                                flush_attn(*pend.pop(0))
                            if k in po_at:
                                emit_outproj_nk(4 * (qc - 1) + po_at[k])
                        for e in pend:
                            flush_attn(*e)

                        # gating: A = ao * (tanh(g/2)+1) / dn  (0.5 in Wo)
                        dnb = gat_p.tile([1, 2, 512], BF, tag="dnb")
                        nc.scalar.activation(out=dnb, in_=aoT[64:65, :]
                                             .rearrange("p (a b) -> p a b", a=2),
                                             func=AF.Copy)
                        rb0 = psX.tile([128, 512], F32, tag="px")
                        nc.tensor.matmul(rb0[0:64, :], halves[0:1, :],
                                         dnb[:, 0, :], start=True, stop=True)
                        rb1 = psX.tile([128, 512], F32, tag="px")
                        nc.tensor.matmul(rb1[0:64, :], halves[0:1, :],
                                         dnb[:, 1, :], start=True, stop=True)
                        rec0 = gat_p.tile([64, 512], F32, tag="rec0")
                        nc.vector.reciprocal_approx_fast(out=rec0,
                                                         in_=rb0[0:64, :])
                        m0 = gat_p.tile([64, 512], F32, tag="m0")
                        nc.vector.scalar_tensor_tensor(
                            out=m0, in0=gs[0:64, :], scalar=1.0,
                            in1=rec0, op0=OP.add, op1=OP.mult)
                        nc.vector.tensor_tensor(out=A_sb[0:64, p, qsl],
                                                in0=aoA[0:64, :], in1=m0,
                                                op=OP.mult)
                        rec1 = gat_p.tile([64, 512], F32, tag="rec1")
                        nc.vector.reciprocal_approx_fast(out=rec1,
                                                         in_=rb1[0:64, :])
                        m1 = gat_p.tile([64, 512], F32, tag="m1")
                        nc.vector.scalar_tensor_tensor(
                            out=m1, in0=gs1, scalar=1.0,
                            in1=rec1, op0=OP.add, op1=OP.mult)
                        At = gat_p.tile([64, 512], BF, tag="At")
                        nc.vector.tensor_tensor(out=At, in0=aoB[0:64, :],
                                                in1=m1, op=OP.mult)
                        nc.sync.dma_start(out=A_sb[64:128, p, qsl], in_=At)

                for nk in range(12, 16):
                    emit_outproj_nk(nk, tail=True)

    nc.compile()
    return nc


def _prep_core(inputs, b, g, bf16):
    x = np.asarray(inputs["x"][b], dtype=np.float32)
    ctx = np.asarray(inputs["context"][b], dtype=np.float32)
    Wq = np.asarray(inputs["Wq"], dtype=np.float32).reshape(H, 2 * D, C)
    Wkv = np.asarray(inputs["Wkv"], dtype=np.float32).reshape(H, 2 * D, C)
    Wo = np.asarray(inputs["Wo"], dtype=np.float32)
    cos = np.asarray(inputs["cos"][b], dtype=np.float32)
    sin = np.asarray(inputs["sin"][b], dtype=np.float32)
    qw = np.asarray(inputs["q_norm_w"], dtype=np.float32)
    kw = np.asarray(inputs["k_norm_w"], dtype=np.float32)

    hs = slice(HG * g, HG * g + HG)
    qr = Wq[hs, :D, :]
    qr = qr - qr.mean(axis=1, keepdims=True)
    gr = Wq[hs, D:, :]
    kr = Wkv[hs, :D, :]
    kr = kr - kr.mean(axis=1, keepdims=True)
    vr = Wkv[hs, D:, :]

    sgn = np.where(np.arange(D) < D // 2, -1.0, 1.0).astype(np.float32)
    wswap = lambda w: np.concatenate([w[D // 2:], w[:D // 2]])

    def pmajor(w, cols):
        # [cols, C] weight -> transposed, partition-major [128, 8, cols]
        return np.ascontiguousarray(
            w.reshape(cols, C).T.reshape(8, 128, cols).transpose(1, 0, 2))

    def tokmajor(t):
        # [C, N] -> [128, 4, 8, 512]: partition, q-block, c-chunk, token
        return np.ascontiguousarray(
            t.reshape(8, 128, 4, 512).transpose(1, 2, 0, 3))

    def tabs(t):
        # [N, D] -> [128, 16, D] bf16
        return np.ascontiguousarray(
            t.reshape(16, 128, D).transpose(1, 0, 2)).astype(bf16)

    return {
        "xT": tokmajor(x.T).astype(bf16),
        "ctxT": tokmajor(ctx.T).astype(bf16),
        "wq": pmajor(qr, 256).astype(bf16),
        "wg": pmajor(gr, 256).astype(bf16),
        "wkv": pmajor(
            np.concatenate([kr.reshape(HG * D, C), vr.reshape(HG * D, C)], 0),
            512).astype(bf16),
        "wo": np.ascontiguousarray(
            0.5 * Wo[:, 256 * g:256 * (g + 1)].T.reshape(2, 128, C)
            .transpose(1, 0, 2)).astype(bf16),
        "cosq": tabs(cos * qw[None, :]),
        "ssinq": tabs(sin * sgn[None, :] * wswap(qw)[None, :]),
        "cosk": tabs(cos * kw[None, :]),
        "ssink": tabs(sin * sgn[None, :] * wswap(kw)[None, :]),
    }


def kernel(**inputs):
    global _PROG, LAST_EXEC_NS, LAST_PROFILE
    import ml_dtypes
    bf16 = ml_dtypes.bfloat16

    if _PROG is None:
        _PROG = _build_program()
    nc = _PROG

    in_maps = [_prep_core(inputs, core // 4, core % 4, bf16) for core in range(8)]

    trace = bool(os.environ.get("BASS_KERNEL_TRACE"))
    kw = {}
    if trace:
        import types
        from trn_agent_boot.trn_boot import _ntff_profile_via_ctypes
        hook = _ntff_profile_via_ctypes('/opt/axon/libaxon_pjrt.so')
        mod = types.ModuleType('antenv.axon_hooks')
        mod.get_axon_ntff_profile_hook = lambda: hook
        sys.modules['antenv.axon_hooks'] = mod
        from concourse import bass_utils
        bass_utils.upload_artifacts = lambda tmpdir: tmpdir
        kw = dict(trace=True, tmpdir=os.environ.get("BASS_KERNEL_TRACE_DIR"))

    from concourse.bass_utils import run_bass_kernel_spmd
    res = run_bass_kernel_spmd(nc, in_maps, core_ids=list(range(8)), **kw)
    LAST_EXEC_NS = res.exec_time_ns
    LAST_PROFILE = res.profile_json

    bo = np.asarray(inputs["bo"], dtype=np.float32)
    out = np.zeros((B, N, C), dtype=np.float32)
    for core in range(8):
        out[core // 4] += np.asarray(res.results[core]["part"],
                                     dtype=np.float32)
    out += bo[None, None, :]
    return out
